# revision 23
# baseline (speedup 1.0000x reference)
"""Trainium2 Bass kernel for CrossVoxAttentionL2.

Sharding: 8 cores = 4 batches x 2 query-halves. Each core computes the full
attention (all 8 heads) for its 2048 queries of one batch; descriptor K/V are
computed per-core (replicated across the 2 cores of a batch).

Device math (per core, transposed-score layout):
  embT  = LN(conv-patch matmul) + pos              [64, 2048]
  desT  = descriptor @ wd + bd                     [64, 1024]
  per head h:
    qT_aug [66, 2048]: rows 0:64 = q, 64 = |q|^2, 65 = 1
    kT_aug [66, 1024]: rows 0:64 = 2k, 64 = -1, 65 = -|k|^2 + ln(alpha*SCALE)
    x[k, q] = kT_aug.T-contract-qT_aug = ln(a*s) - |q-k|^2     (one K=66 matmul)
    u = exp(x); p = exp(u)                         (two ScalarE passes)
    attnT[d, q] (+ rowsum row via ones-column of v) = v_aug.T @ p
    attn_n = attnT * (1/rowsum)  -> fp16
  outT = wo.T-contract-attn_n + bo                 [64, 2048]
"""

import os
import sys
from contextlib import ExitStack

import numpy as np

for _p in ("/opt/trn_rl_repo", "/root/.axon_site/_ro/trn_rl_repo"):
    if os.path.isdir(_p) and _p not in sys.path:
        sys.path.insert(0, _p)

import concourse.bass as bass
import concourse.mybir as mybir
import concourse.tile as tile
from concourse import bacc
from concourse.bass_utils import run_bass_kernel_spmd
from concourse.masks import make_identity

F16 = mybir.dt.float16
F32 = mybir.dt.float32
AX = mybir.AxisListType
OP = mybir.AluOpType
ACT = mybir.ActivationFunctionType

B, V, P, E, H, DSC, SD = 4, 64, 4, 64, 8, 256, 1024
NP_ = (V // P) ** 3          # 4096 patches
Q = NP_ // 2                 # 2048 queries per core
SCALE = np.float32(1.0 / np.sqrt(E))
LN_EPS = 1e-5
NCORES = 8

# per-core dram tensor specs: name -> (shape, dtype)
_IN_SPECS = {
    "xt":    ((64, Q), F16),          # patch voxels x queries
    "cwt":   ((65, 64), F16),         # conv weight (vox, e) + conv_b row
    "desct": ((2, 128, SD), F16),     # descriptor.T chunked on dsc
    "wdc":   ((2, 128, 64), F16),     # wd chunked on dsc
    "bd":    ((64, 1), F32),
    "wq":    ((65, 512), F16),        # wq + bq row
    "wk2":   ((65, 512), F16),        # 2*wk + 2*bk row
    "wv":    ((65, 512), F16),        # wv + bv row
    "wo":    ((64, 8, 64), F16),      # wo per head: [d, h, c]
    "bo":    ((64, 1), F32),
    "lng":   ((128, 64), F32),        # ln_g broadcast over partitions
    "bpos":  ((128, 16, 64), F32),    # ln_b + pos_emb slice, [p, qc, e]
    "lnc":   ((1, 1), F32),           # ln(alpha * SCALE)
    "negs":  ((1, SD), F16),          # constant -1 row
    "onesq": ((1, Q), F16),           # constant +1 row
}
_OUT_NAME = "outt"                    # (64, Q) f32, transposed output


def _emit(ctx: ExitStack, tc: tile.TileContext, io: dict):
    nc = tc.nc
    wp = ctx.enter_context(tc.tile_pool(name="wp", bufs=1))
    work = ctx.enter_context(tc.tile_pool(name="work", bufs=3))
    work2 = ctx.enter_context(tc.tile_pool(name="work2", bufs=2))
    work1 = ctx.enter_context(tc.tile_pool(name="work1", bufs=1))
    pers = ctx.enter_context(tc.tile_pool(name="pers", bufs=1))

    def load(name, shape=None, dt=None, rearrange=None):
        shp, dtt = _IN_SPECS[name]
        t = wp.tile(list(shape or shp), dt or dtt, tag=name, name=name)
        src = io[name]
        if rearrange:
            src = src.rearrange(rearrange)
        nc.sync.dma_start(t[:], src)
        return t

    cwt = load("cwt")
    desct = load("desct", shape=(128, 2, SD), rearrange="c p s -> p c s")
    wdc = load("wdc", shape=(128, 2, 64), rearrange="c p s -> p c s")
    bd = load("bd")
    wq = load("wq")
    wk2 = load("wk2")
    wv = load("wv")
    wo = load("wo")
    bo = load("bo")
    lng = load("lng")
    bpos = load("bpos")
    lnc = load("lnc")

    xt = wp.tile([65, Q], F16, tag="xt")
    nc.sync.dma_start(xt[:64, :], io["xt"])
    nc.sync.dma_start(xt[64:65, :], io["onesq"])

    ident = wp.tile([128, 128], F16, tag="ident")
    make_identity(nc, ident[:])
    ones64 = wp.tile([64, 1], F16, tag="ones64")
    nc.vector.memset(ones64[:], 1.0)

    det = pers.tile([65, SD], F16, tag="det")
    emb = pers.tile([128, 16, 64], F32, tag="emb")
    vemb = pers.tile([128, 16, 64], F16, tag="vemb")
    vet = pers.tile([65, Q], F16, tag="vet")
    vsb = pers.tile([128, 8, 8, 66], F16, tag="vsb")
    qts = [pers.tile([66, Q], F16, tag=f"qt{h}", name=f"qt{h}") for h in range(H)]
    kts = [pers.tile([66, SD], F16, tag=f"kt{h}", name=f"kt{h}") for h in range(H)]
    attn = [pers.tile([66, Q], F16, tag=f"attn{i}", name=f"attn{i}")
            for i in range(H)]
    outt = pers.tile([64, Q], F32, tag="outt")

    # ================= phase A: des_emb, conv, LN, transpose =================
    with tc.tile_pool(name="psA", bufs=2, space="PSUM") as psA:
        ps_de = psA.tile([64, SD], F32, tag="de")
        for sc in range(2):
            for kc in range(2):
                nc.tensor.matmul(
                    ps_de[:, sc * 512:(sc + 1) * 512],
                    lhsT=wdc[:, kc, :], rhs=desct[:, kc, sc * 512:(sc + 1) * 512],
                    start=(kc == 0), stop=(kc == 1))
        nc.scalar.activation(det[:64, :], ps_de[:], ACT.Identity, bias=bd[:])
        nc.sync.dma_start(det[64:65, :], io["onesq"][:, :SD])

        for g in range(4):  # 4 groups of 4 q-chunks of 128
            ps_e = psA.tile([128, 4, 64], F32, tag="e")
            for j in range(4):
                i = g * 4 + j
                nc.tensor.matmul(ps_e[:, j, :], lhsT=xt[:, i * 128:(i + 1) * 128],
                                 rhs=cwt[:], start=True, stop=True)
            nc.vector.tensor_copy(emb[:, g * 4:(g + 1) * 4, :], ps_e[:])

        s1 = work1.tile([128, 16], F32, tag="s1")
        s2 = work1.tile([128, 16], F32, tag="s2")
        sqe = work1.tile([128, 16, 64], F32, tag="sqe")
        nc.scalar.activation(sqe[:], emb[:], ACT.Square)
        nc.vector.tensor_reduce(s1[:], emb[:], axis=AX.X, op=OP.add)
        nc.vector.tensor_reduce(s2[:], sqe[:], axis=AX.X, op=OP.add)
        mu = work1.tile([128, 16], F32, tag="mu")
        nc.vector.tensor_scalar_mul(mu[:], s1[:], 1.0 / 64)
        var = work1.tile([128, 16], F32, tag="var")
        nc.vector.tensor_scalar_mul(var[:], s2[:], 1.0 / 64)
        musq = work1.tile([128, 16], F32, tag="musq")
        nc.vector.tensor_tensor(musq[:], mu[:], mu[:], op=OP.mult)
        nc.vector.tensor_tensor(var[:], var[:], musq[:], op=OP.subtract)
        nc.vector.tensor_scalar_add(var[:], var[:], LN_EPS)
        std = work1.tile([128, 16], F32, tag="std")
        nc.scalar.activation(std[:], var[:], ACT.Sqrt)
        rstd = work1.tile([128, 16], F32, tag="rstd")
        nc.vector.reciprocal(rstd[:], std[:])

        t1 = work1.tile([128, 16, 64], F32, tag="t1")
        nc.vector.tensor_tensor(
            t1[:], emb[:], mu[:, :, None].to_broadcast((128, 16, 64)),
            op=OP.subtract)
        nc.vector.tensor_tensor(
            t1[:], t1[:], rstd[:, :, None].to_broadcast((128, 16, 64)), op=OP.mult)
        nc.vector.tensor_tensor(
            t1[:], t1[:], lng[:, None, :].to_broadcast((128, 16, 64)), op=OP.mult)
        nc.vector.tensor_tensor(vemb[:], t1[:], bpos[:], op=OP.add)

        for qc in range(4):
            ps_t = psA.tile([64, 512], F16, tag="tr")
            for j in range(4):
                i = qc * 4 + j
                nc.tensor.transpose(ps_t[:, j * 128:(j + 1) * 128], vemb[:, i, :],
                                    ident[:])
            nc.vector.tensor_copy(vet[:64, qc * 512:(qc + 1) * 512], ps_t[:])
        nc.sync.dma_start(vet[64:65, :], io["onesq"])

    # ================= phase B: per-head q/k/v prep =================
    nc.gpsimd.memset(vsb[:, :, :, 64:65], 1.0)
    with tc.tile_pool(name="psB", bufs=1, space="PSUM") as psB:
        for h in range(H):
            qt, kt = qts[h], kts[h]
            ps_q = psB.tile([128, Q], F32, tag="q")
            for qc in range(4):
                nc.tensor.matmul(ps_q[:64, qc * 512:(qc + 1) * 512],
                                 lhsT=wq[:, h * 64:(h + 1) * 64],
                                 rhs=vet[:, qc * 512:(qc + 1) * 512],
                                 start=True, stop=True)
            nc.vector.tensor_copy(qt[:64, :], ps_q[:64, :])
            sq = work2.tile([64, Q], F16, tag="sq_q")
            nc.scalar.activation(sq[:], ps_q[:64, :], ACT.Square)
            for qc in range(4):
                ps_q2 = psB.tile([128, 512], F32, tag="sm")
                nc.tensor.matmul(ps_q2[:1, :], lhsT=ones64[:],
                                 rhs=sq[:, qc * 512:(qc + 1) * 512],
                                 start=True, stop=True)
                nc.vector.tensor_copy(qt[64:65, qc * 512:(qc + 1) * 512],
                                      ps_q2[:1, :])
            nc.sync.dma_start(qt[65:66, :], io["onesq"])

            ps_k = psB.tile([128, SD], F32, tag="k")
            for sc in range(2):
                nc.tensor.matmul(ps_k[:64, sc * 512:(sc + 1) * 512],
                                 lhsT=wk2[:, h * 64:(h + 1) * 64],
                                 rhs=det[:, sc * 512:(sc + 1) * 512],
                                 start=True, stop=True)
            nc.vector.tensor_copy(kt[:64, :], ps_k[:64, :])
            sqk = work2.tile([64, SD], F16, tag="sq_k")
            nc.scalar.activation(sqk[:], ps_k[:64, :], ACT.Square)
            k2row = work2.tile([1, SD], F16, tag="k2row")
            for sc in range(2):
                ps_k2 = psB.tile([128, 512], F32, tag="sm")
                nc.tensor.matmul(ps_k2[:1, :], lhsT=ones64[:],
                                 rhs=sqk[:, sc * 512:(sc + 1) * 512],
                                 start=True, stop=True)
                nc.vector.tensor_scalar(k2row[:, sc * 512:(sc + 1) * 512],
                                        ps_k2[:1, :], -0.25, lnc[:],
                                        op0=OP.mult, op1=OP.add)
            nc.sync.dma_start(kt[64:65, :], io["negs"])
            nc.sync.dma_start(kt[65:66, :], k2row[:])

            ps_v = psB.tile([128, 512], F32, tag="v")
            nc.tensor.matmul(ps_v[:], lhsT=det[:, h * 128:(h + 1) * 128], rhs=wv[:],
                             start=True, stop=True)
            nc.any.tensor_copy(vsb[:, h, :, 0:64], ps_v[:])

    # ================= phase C: attention main loop + out-proj =================
    with tc.tile_pool(name="psC", bufs=1, space="PSUM") as psC:
        for h in range(H):
            at = psC.tile([128, Q], F32, tag="at")
            for kc in range(8):
                x = psC.tile([128, Q], F32, tag="x")
                for qc in range(4):
                    nc.tensor.matmul(x[:, qc * 512:(qc + 1) * 512],
                                     lhsT=kts[h][:, kc * 128:(kc + 1) * 128],
                                     rhs=qts[h][:, qc * 512:(qc + 1) * 512],
                                     start=True, stop=True)
                u = work.tile([128, Q], F16, tag="u")
                nc.scalar.activation(u[:], x[:], ACT.Exp)
                p = work.tile([128, Q], F16, tag="p")
                nc.scalar.activation(p[:], u[:], ACT.Exp)
                for qc in range(4):
                    nc.tensor.matmul(at[:65, qc * 512:(qc + 1) * 512],
                                     lhsT=vsb[:, kc, h, 0:65],
                                     rhs=p[:, qc * 512:(qc + 1) * 512],
                                     start=(kc == 0), stop=(kc == 7))
            nc.vector.tensor_copy(attn[h][0:65, :], at[0:65, :])
            srec = work2.tile([1, Q], F16, tag="srec")
            with nc.allow_low_precision(reason="1/S fp16: S~1e3, rel 5e-4 ok"):
                nc.vector.reciprocal(srec[:], attn[h][64:65, :])
            recb = work2.tile([64, Q], F16, tag="recb")
            nc.gpsimd.partition_broadcast(recb[:], srec[:])
            nc.vector.tensor_tensor(attn[h][0:64, :], attn[h][0:64, :], recb[:],
                                    op=OP.mult)

        ps_o = psC.tile([128, Q], F32, tag="at")
        for h in range(H):
            for qc in range(4):
                nc.tensor.matmul(ps_o[:64, qc * 512:(qc + 1) * 512],
                                 lhsT=wo[:, h, :],
                                 rhs=attn[h][0:64, qc * 512:(qc + 1) * 512],
                                 start=(h == 0), stop=(h == 7))
        nc.scalar.activation(outt[:], ps_o[:64, :], ACT.Identity, bias=bo[:])
    nc.sync.dma_start(io[_OUT_NAME], outt[:])


def build_module():
    nc = bacc.Bacc("TRN2", target_bir_lowering=False, debug=False,
                   enable_asserts=False, num_devices=NCORES)
    io = {}
    for name, (shape, dt) in _IN_SPECS.items():
        io[name] = nc.dram_tensor(name, list(shape), dt, kind="ExternalInput").ap()
    io[_OUT_NAME] = nc.dram_tensor(_OUT_NAME, [64, Q], F32, kind="ExternalOutput").ap()
    with tile.TileContext(nc) as tc:
        with ExitStack() as ctx:
            _emit(ctx, tc, io)
    nc.compile()
    return nc


def shard_inputs(inputs) -> list:
    v = {k: np.asarray(val) for k, val in inputs.items()}
    f32 = np.float32
    f16 = np.float16
    cw = v["conv_w"].reshape(E, 64).astype(f32)
    cwt = np.concatenate([cw.T, v["conv_b"][None, :]], 0).astype(f16)     # [65, 64]
    wdc = np.ascontiguousarray(v["wd"].reshape(2, 128, 64)).astype(f16)   # [2,128,64]
    wq = np.concatenate([v["wq"], v["bq"][None, :]], 0).astype(f16)       # [65, 512]
    wk2 = np.concatenate([2.0 * v["wk"], 2.0 * v["bk"][None, :]], 0).astype(f16)
    wv = np.concatenate([v["wv"], v["bv"][None, :]], 0).astype(f16)
    wo = np.ascontiguousarray(
        v["wo"].reshape(8, 64, 64).transpose(1, 0, 2)).astype(f16)   # [d, h, c]
    bo = v["bo"].reshape(64, 1).astype(f32)
    bd = v["bd"].reshape(64, 1).astype(f32)
    lng = np.broadcast_to(v["ln_g"], (128, 64)).astype(f32).copy()
    lnc = np.array([[np.log(np.float32(v["alpha"]) * SCALE)]], dtype=f32)

    maps = []
    for c in range(NCORES):
        b, qh = c // 2, c % 2
        vox = v["vox_features"][b, 0].reshape(16, 4, 16, 4, 16, 4)
        X = vox.transpose(0, 2, 4, 1, 3, 5).reshape(NP_, 64)[qh * Q:(qh + 1) * Q]
        xt = np.ascontiguousarray(X.T).astype(f16)                        # [64, Q]
        desct = np.ascontiguousarray(v["descriptor"][b].T).reshape(2, 128, SD).astype(f16)
        bp = (v["ln_b"][None, :] + v["pos_emb"][0, qh * Q:(qh + 1) * Q, :]).astype(f32)
        bpos = np.ascontiguousarray(bp.reshape(16, 128, 64).transpose(1, 0, 2))
        maps.append({
            "xt": xt, "cwt": cwt, "desct": desct, "wdc": wdc, "bd": bd,
            "wq": wq, "wk2": wk2, "wv": wv, "wo": wo, "bo": bo,
            "lng": lng, "bpos": bpos, "lnc": lnc,
            "negs": np.full((1, SD), -1.0, np.float16),
            "onesq": np.ones((1, Q), np.float16),
        })
    return maps


def assemble_output(results) -> np.ndarray:
    out = np.zeros((B, NP_, 64), np.float32)
    for c in range(NCORES):
        b, qh = c // 2, c % 2
        out[b, qh * Q:(qh + 1) * Q, :] = results[c][_OUT_NAME].T
    return out


_NC_CACHE = {}


def kernel(**inputs) -> np.ndarray:
    if "nc" not in _NC_CACHE:
        _NC_CACHE["nc"] = build_module()
    nc = _NC_CACHE["nc"]
    maps = shard_inputs(inputs)
    res = run_bass_kernel_spmd(nc, maps, core_ids=list(range(NCORES)))
    return assemble_output(res.results)


# revision 24
# speedup vs baseline: 1.0342x; 1.0342x over previous
"""Trainium2 Bass kernel for CrossVoxAttentionL2.

Sharding: 8 cores = 4 batches x 2 query-halves. Each core computes the full
attention (all 8 heads) for its 2048 queries of one batch; descriptor K/V are
computed per-core (replicated across the 2 cores of a batch).

Device math (per core, transposed-score layout):
  embT  = LN(conv-patch matmul) + pos              [64, 2048]
  desT  = descriptor @ wd + bd                     [64, 1024]
  per head h:
    qT_aug [66, 2048]: rows 0:64 = q, 64 = |q|^2, 65 = 1
    kT_aug [66, 1024]: rows 0:64 = 2k, 64 = -1, 65 = -|k|^2 + ln(alpha*SCALE)
    x[k, q] = kT_aug.T-contract-qT_aug = ln(a*s) - |q-k|^2     (one K=66 matmul)
    u = exp(x); p = exp(u)                         (two ScalarE passes)
    attnT[d, q] (+ rowsum row via ones-column of v) = v_aug.T @ p
    attn_n = attnT * (1/rowsum)  -> fp16
  outT = wo.T-contract-attn_n + bo                 [64, 2048]
"""

import os
import sys
from contextlib import ExitStack

import numpy as np

for _p in ("/opt/trn_rl_repo", "/root/.axon_site/_ro/trn_rl_repo"):
    if os.path.isdir(_p) and _p not in sys.path:
        sys.path.insert(0, _p)

import concourse.bass as bass
import concourse.mybir as mybir
import concourse.tile as tile
from concourse import bacc
from concourse.bass_utils import run_bass_kernel_spmd
from concourse.masks import make_identity

F16 = mybir.dt.float16
F32 = mybir.dt.float32
AX = mybir.AxisListType
OP = mybir.AluOpType
ACT = mybir.ActivationFunctionType

B, V, P, E, H, DSC, SD = 4, 64, 4, 64, 8, 256, 1024
NP_ = (V // P) ** 3          # 4096 patches
Q = NP_ // 2                 # 2048 queries per core
SCALE = np.float32(1.0 / np.sqrt(E))
LN_EPS = 1e-5
NCORES = 8

# per-core dram tensor specs: name -> (shape, dtype)
_IN_SPECS = {
    "xt":    ((64, Q), F16),          # patch voxels x queries
    "cwt":   ((65, 64), F16),         # conv weight (vox, e) + conv_b row
    "desct": ((2, 128, SD), F16),     # descriptor.T chunked on dsc
    "wdc":   ((2, 128, 64), F16),     # wd chunked on dsc
    "bd":    ((64, 1), F32),
    "wq":    ((65, 512), F16),        # wq + bq row
    "wk2":   ((65, 512), F16),        # 2*wk + 2*bk row
    "wv":    ((65, 512), F16),        # wv + bv row
    "wo":    ((64, 8, 64), F16),      # wo per head: [d, h, c]
    "bo":    ((64, 1), F32),
    "lng":   ((128, 64), F32),        # ln_g broadcast over partitions
    "bpos":  ((128, 16, 64), F32),    # ln_b + pos_emb slice, [p, qc, e]
    "lnc":   ((1, 1), F32),           # ln(alpha * SCALE)
    "negs":  ((1, SD), F16),          # constant -1 row
    "onesq": ((1, Q), F16),           # constant +1 row
}
_OUT_NAME = "outt"                    # (64, Q) f32, transposed output


def _emit(ctx: ExitStack, tc: tile.TileContext, io: dict):
    nc = tc.nc
    wp = ctx.enter_context(tc.tile_pool(name="wp", bufs=1))
    work = ctx.enter_context(tc.tile_pool(name="work", bufs=3))
    work2 = ctx.enter_context(tc.tile_pool(name="work2", bufs=2))
    work1 = ctx.enter_context(tc.tile_pool(name="work1", bufs=1))
    pers = ctx.enter_context(tc.tile_pool(name="pers", bufs=1))

    def load(name, shape=None, dt=None, rearrange=None):
        shp, dtt = _IN_SPECS[name]
        t = wp.tile(list(shape or shp), dt or dtt, tag=name, name=name)
        src = io[name]
        if rearrange:
            src = src.rearrange(rearrange)
        nc.sync.dma_start(t[:], src)
        return t

    cwt = load("cwt")
    desct = load("desct", shape=(128, 2, SD), rearrange="c p s -> p c s")
    wdc = load("wdc", shape=(128, 2, 64), rearrange="c p s -> p c s")
    bd = load("bd")
    wq = load("wq")
    wk2 = load("wk2")
    wv = load("wv")
    wo = load("wo")
    bo = load("bo")
    lng = load("lng")
    bpos = load("bpos")
    lnc = load("lnc")

    xt = wp.tile([65, Q], F16, tag="xt")
    nc.sync.dma_start(xt[:64, :], io["xt"])
    nc.sync.dma_start(xt[64:65, :], io["onesq"])

    ident = wp.tile([128, 128], F16, tag="ident")
    make_identity(nc, ident[:])
    ones64 = wp.tile([64, 1], F16, tag="ones64")
    nc.vector.memset(ones64[:], 1.0)

    det = pers.tile([65, SD], F16, tag="det")
    emb = pers.tile([128, 16, 64], F32, tag="emb")
    vemb = pers.tile([128, 16, 64], F16, tag="vemb")
    vet = pers.tile([65, Q], F16, tag="vet")
    vsb = pers.tile([128, 8, 8, 66], F16, tag="vsb")
    qts = [pers.tile([66, Q], F16, tag=f"qt{h}", name=f"qt{h}") for h in range(H)]
    kts = [pers.tile([66, SD], F16, tag=f"kt{h}", name=f"kt{h}") for h in range(H)]
    attn = [pers.tile([66, Q], F16, tag=f"attn{i}", name=f"attn{i}")
            for i in range(H)]
    outt = pers.tile([64, Q], F32, tag="outt")

    # ================= phase A: des_emb, conv, LN, transpose =================
    with tc.tile_pool(name="psA", bufs=2, space="PSUM") as psA:
        ps_de = psA.tile([64, SD], F32, tag="de")
        for sc in range(2):
            for kc in range(2):
                nc.tensor.matmul(
                    ps_de[:, sc * 512:(sc + 1) * 512],
                    lhsT=wdc[:, kc, :], rhs=desct[:, kc, sc * 512:(sc + 1) * 512],
                    start=(kc == 0), stop=(kc == 1))
        nc.scalar.activation(det[:64, :], ps_de[:], ACT.Identity, bias=bd[:])
        nc.sync.dma_start(det[64:65, :], io["onesq"][:, :SD])

        for g in range(4):  # 4 groups of 4 q-chunks of 128
            ps_e = psA.tile([128, 4, 64], F32, tag="e")
            for j in range(4):
                i = g * 4 + j
                nc.tensor.matmul(ps_e[:, j, :], lhsT=xt[:, i * 128:(i + 1) * 128],
                                 rhs=cwt[:], start=True, stop=True)
            nc.vector.tensor_copy(emb[:, g * 4:(g + 1) * 4, :], ps_e[:])

        s1 = work1.tile([128, 16], F32, tag="s1")
        s2 = work1.tile([128, 16], F32, tag="s2")
        sqe = work1.tile([128, 16, 64], F32, tag="sqe")
        nc.scalar.activation(sqe[:], emb[:], ACT.Square)
        nc.vector.tensor_reduce(s1[:], emb[:], axis=AX.X, op=OP.add)
        nc.vector.tensor_reduce(s2[:], sqe[:], axis=AX.X, op=OP.add)
        mu = work1.tile([128, 16], F32, tag="mu")
        nc.vector.tensor_scalar_mul(mu[:], s1[:], 1.0 / 64)
        var = work1.tile([128, 16], F32, tag="var")
        nc.vector.tensor_scalar_mul(var[:], s2[:], 1.0 / 64)
        musq = work1.tile([128, 16], F32, tag="musq")
        nc.vector.tensor_tensor(musq[:], mu[:], mu[:], op=OP.mult)
        nc.vector.tensor_tensor(var[:], var[:], musq[:], op=OP.subtract)
        nc.vector.tensor_scalar_add(var[:], var[:], LN_EPS)
        std = work1.tile([128, 16], F32, tag="std")
        nc.scalar.activation(std[:], var[:], ACT.Sqrt)
        rstd = work1.tile([128, 16], F32, tag="rstd")
        nc.vector.reciprocal(rstd[:], std[:])

        t1 = work1.tile([128, 16, 64], F32, tag="t1")
        nc.vector.tensor_tensor(
            t1[:], emb[:], mu[:, :, None].to_broadcast((128, 16, 64)),
            op=OP.subtract)
        nc.vector.tensor_tensor(
            t1[:], t1[:], rstd[:, :, None].to_broadcast((128, 16, 64)), op=OP.mult)
        nc.vector.tensor_tensor(
            t1[:], t1[:], lng[:, None, :].to_broadcast((128, 16, 64)), op=OP.mult)
        nc.vector.tensor_tensor(vemb[:], t1[:], bpos[:], op=OP.add)

        for qc in range(4):
            ps_t = psA.tile([64, 512], F16, tag="tr")
            for j in range(4):
                i = qc * 4 + j
                nc.tensor.transpose(ps_t[:, j * 128:(j + 1) * 128], vemb[:, i, :],
                                    ident[:])
            nc.vector.tensor_copy(vet[:64, qc * 512:(qc + 1) * 512], ps_t[:])
        nc.sync.dma_start(vet[64:65, :], io["onesq"])

    # ================= phase B: per-head q/k/v prep =================
    nc.gpsimd.memset(vsb[:, :, :, 64:65], 1.0)
    with tc.tile_pool(name="psB", bufs=1, space="PSUM") as psB:
        for h in range(H):
            qt, kt = qts[h], kts[h]
            ps_q = psB.tile([128, Q], F32, tag="q")
            for qc in range(4):
                nc.tensor.matmul(ps_q[:64, qc * 512:(qc + 1) * 512],
                                 lhsT=wq[:, h * 64:(h + 1) * 64],
                                 rhs=vet[:, qc * 512:(qc + 1) * 512],
                                 start=True, stop=True)
            nc.scalar.copy(qt[:64, :], ps_q[:64, :])
            sq = work2.tile([64, Q], F16, tag="sq_q")
            nc.scalar.activation(sq[:], ps_q[:64, :], ACT.Square)
            for qc in range(4):
                ps_q2 = psB.tile([128, 512], F32, tag="sm")
                nc.tensor.matmul(ps_q2[:1, :], lhsT=ones64[:],
                                 rhs=sq[:, qc * 512:(qc + 1) * 512],
                                 start=True, stop=True)
                nc.vector.tensor_copy(qt[64:65, qc * 512:(qc + 1) * 512],
                                      ps_q2[:1, :])
            nc.sync.dma_start(qt[65:66, :], io["onesq"])

            ps_k = psB.tile([128, SD], F32, tag="k")
            for sc in range(2):
                nc.tensor.matmul(ps_k[:64, sc * 512:(sc + 1) * 512],
                                 lhsT=wk2[:, h * 64:(h + 1) * 64],
                                 rhs=det[:, sc * 512:(sc + 1) * 512],
                                 start=True, stop=True)
            nc.scalar.copy(kt[:64, :], ps_k[:64, :])
            sqk = work2.tile([64, SD], F16, tag="sq_k")
            nc.scalar.activation(sqk[:], ps_k[:64, :], ACT.Square)
            k2row = work2.tile([1, SD], F16, tag="k2row")
            for sc in range(2):
                ps_k2 = psB.tile([128, 512], F32, tag="sm")
                nc.tensor.matmul(ps_k2[:1, :], lhsT=ones64[:],
                                 rhs=sqk[:, sc * 512:(sc + 1) * 512],
                                 start=True, stop=True)
                nc.vector.tensor_scalar(k2row[:, sc * 512:(sc + 1) * 512],
                                        ps_k2[:1, :], -0.25, lnc[:],
                                        op0=OP.mult, op1=OP.add)
            nc.sync.dma_start(kt[64:65, :], io["negs"])
            nc.sync.dma_start(kt[65:66, :], k2row[:])

            ps_v = psB.tile([128, 512], F32, tag="v")
            nc.tensor.matmul(ps_v[:], lhsT=det[:, h * 128:(h + 1) * 128], rhs=wv[:],
                             start=True, stop=True)
            nc.any.tensor_copy(vsb[:, h, :, 0:64], ps_v[:])

    # ================= phase C: attention main loop + out-proj =================
    with tc.tile_pool(name="psC", bufs=1, space="PSUM") as psC:
        for h in range(H):
            at = psC.tile([128, Q], F32, tag="at")
            for kc in range(8):
                x = psC.tile([128, Q], F32, tag="x")
                for qc in range(4):
                    nc.tensor.matmul(x[:, qc * 512:(qc + 1) * 512],
                                     lhsT=kts[h][:, kc * 128:(kc + 1) * 128],
                                     rhs=qts[h][:, qc * 512:(qc + 1) * 512],
                                     start=True, stop=True)
                u = work.tile([128, Q], F16, tag="u")
                nc.scalar.activation(u[:], x[:], ACT.Exp)
                p = work.tile([128, Q], F16, tag="p")
                nc.scalar.activation(p[:], u[:], ACT.Exp)
                for qc in range(4):
                    nc.tensor.matmul(at[:65, qc * 512:(qc + 1) * 512],
                                     lhsT=vsb[:, kc, h, 0:65],
                                     rhs=p[:, qc * 512:(qc + 1) * 512],
                                     start=(kc == 0), stop=(kc == 7))
            nc.vector.tensor_copy(attn[h][0:65, :], at[0:65, :])
            srec = work2.tile([1, Q], F16, tag="srec")
            with nc.allow_low_precision(reason="1/S fp16: S~1e3, rel 5e-4 ok"):
                nc.vector.reciprocal(srec[:], attn[h][64:65, :])
            recb = work2.tile([64, Q], F16, tag="recb")
            nc.gpsimd.partition_broadcast(recb[:], srec[:])
            nc.vector.tensor_tensor(attn[h][0:64, :], attn[h][0:64, :], recb[:],
                                    op=OP.mult)

        ps_o = psC.tile([128, Q], F32, tag="at")
        for h in range(H):
            for qc in range(4):
                nc.tensor.matmul(ps_o[:64, qc * 512:(qc + 1) * 512],
                                 lhsT=wo[:, h, :],
                                 rhs=attn[h][0:64, qc * 512:(qc + 1) * 512],
                                 start=(h == 0), stop=(h == 7))
        nc.scalar.activation(outt[:], ps_o[:64, :], ACT.Identity, bias=bo[:])
    nc.sync.dma_start(io[_OUT_NAME], outt[:])


def build_module():
    nc = bacc.Bacc("TRN2", target_bir_lowering=False, debug=False,
                   enable_asserts=False, num_devices=NCORES)
    io = {}
    for name, (shape, dt) in _IN_SPECS.items():
        io[name] = nc.dram_tensor(name, list(shape), dt, kind="ExternalInput").ap()
    io[_OUT_NAME] = nc.dram_tensor(_OUT_NAME, [64, Q], F32, kind="ExternalOutput").ap()
    with tile.TileContext(nc) as tc:
        with ExitStack() as ctx:
            _emit(ctx, tc, io)
    nc.compile()
    return nc


def shard_inputs(inputs) -> list:
    v = {k: np.asarray(val) for k, val in inputs.items()}
    f32 = np.float32
    f16 = np.float16
    cw = v["conv_w"].reshape(E, 64).astype(f32)
    cwt = np.concatenate([cw.T, v["conv_b"][None, :]], 0).astype(f16)     # [65, 64]
    wdc = np.ascontiguousarray(v["wd"].reshape(2, 128, 64)).astype(f16)   # [2,128,64]
    wq = np.concatenate([v["wq"], v["bq"][None, :]], 0).astype(f16)       # [65, 512]
    wk2 = np.concatenate([2.0 * v["wk"], 2.0 * v["bk"][None, :]], 0).astype(f16)
    wv = np.concatenate([v["wv"], v["bv"][None, :]], 0).astype(f16)
    wo = np.ascontiguousarray(
        v["wo"].reshape(8, 64, 64).transpose(1, 0, 2)).astype(f16)   # [d, h, c]
    bo = v["bo"].reshape(64, 1).astype(f32)
    bd = v["bd"].reshape(64, 1).astype(f32)
    lng = np.broadcast_to(v["ln_g"], (128, 64)).astype(f32).copy()
    lnc = np.array([[np.log(np.float32(v["alpha"]) * SCALE)]], dtype=f32)

    maps = []
    for c in range(NCORES):
        b, qh = c // 2, c % 2
        vox = v["vox_features"][b, 0].reshape(16, 4, 16, 4, 16, 4)
        X = vox.transpose(0, 2, 4, 1, 3, 5).reshape(NP_, 64)[qh * Q:(qh + 1) * Q]
        xt = np.ascontiguousarray(X.T).astype(f16)                        # [64, Q]
        desct = np.ascontiguousarray(v["descriptor"][b].T).reshape(2, 128, SD).astype(f16)
        bp = (v["ln_b"][None, :] + v["pos_emb"][0, qh * Q:(qh + 1) * Q, :]).astype(f32)
        bpos = np.ascontiguousarray(bp.reshape(16, 128, 64).transpose(1, 0, 2))
        maps.append({
            "xt": xt, "cwt": cwt, "desct": desct, "wdc": wdc, "bd": bd,
            "wq": wq, "wk2": wk2, "wv": wv, "wo": wo, "bo": bo,
            "lng": lng, "bpos": bpos, "lnc": lnc,
            "negs": np.full((1, SD), -1.0, np.float16),
            "onesq": np.ones((1, Q), np.float16),
        })
    return maps


def assemble_output(results) -> np.ndarray:
    out = np.zeros((B, NP_, 64), np.float32)
    for c in range(NCORES):
        b, qh = c // 2, c % 2
        out[b, qh * Q:(qh + 1) * Q, :] = results[c][_OUT_NAME].T
    return out


_NC_CACHE = {}


def kernel(**inputs) -> np.ndarray:
    if "nc" not in _NC_CACHE:
        _NC_CACHE["nc"] = build_module()
    nc = _NC_CACHE["nc"]
    maps = shard_inputs(inputs)
    res = run_bass_kernel_spmd(nc, maps, core_ids=list(range(NCORES)))
    return assemble_output(res.results)


# revision 25
# speedup vs baseline: 1.0569x; 1.0220x over previous
"""Trainium2 Bass kernel for CrossVoxAttentionL2.

Sharding: 8 cores = 4 batches x 2 query-halves. Each core computes the full
attention (all 8 heads) for its 2048 queries of one batch; descriptor K/V are
computed per-core (replicated across the 2 cores of a batch).

Device math (per core, transposed-score layout):
  embT  = LN(conv-patch matmul) + pos              [64, 2048]
  desT  = descriptor @ wd + bd                     [64, 1024]
  per head h:
    qT_aug [66, 2048]: rows 0:64 = q, 64 = |q|^2, 65 = 1
    kT_aug [66, 1024]: rows 0:64 = 2k, 64 = -1, 65 = -|k|^2 + ln(alpha*SCALE)
    x[k, q] = kT_aug.T-contract-qT_aug = ln(a*s) - |q-k|^2     (one K=66 matmul)
    u = exp(x); p = exp(u)                         (two ScalarE passes)
    attnT[d, q] (+ rowsum row via ones-column of v) = v_aug.T @ p
    attn_n = attnT * (1/rowsum)  -> fp16
  outT = wo.T-contract-attn_n + bo                 [64, 2048]
"""

import os
import sys
from contextlib import ExitStack

import numpy as np

for _p in ("/opt/trn_rl_repo", "/root/.axon_site/_ro/trn_rl_repo"):
    if os.path.isdir(_p) and _p not in sys.path:
        sys.path.insert(0, _p)

import concourse.bass as bass
import concourse.mybir as mybir
import concourse.tile as tile
from concourse import bacc
from concourse.bass_utils import run_bass_kernel_spmd
from concourse.masks import make_identity

F16 = mybir.dt.float16
F32 = mybir.dt.float32
AX = mybir.AxisListType
OP = mybir.AluOpType
ACT = mybir.ActivationFunctionType

B, V, P, E, H, DSC, SD = 4, 64, 4, 64, 8, 256, 1024
NP_ = (V // P) ** 3          # 4096 patches
Q = NP_ // 2                 # 2048 queries per core
SCALE = np.float32(1.0 / np.sqrt(E))
LN_EPS = 1e-5
NCORES = 8

# per-core dram tensor specs: name -> (shape, dtype)
_IN_SPECS = {
    "xt":    ((64, Q), F16),          # patch voxels x queries
    "cwt":   ((65, 64), F16),         # conv weight (vox, e) + conv_b row
    "desct": ((2, 128, SD), F16),     # descriptor.T chunked on dsc
    "wdc":   ((2, 128, 64), F16),     # wd chunked on dsc
    "bd":    ((64, 1), F32),
    "wq":    ((65, 512), F16),        # wq + bq row
    "wk2":   ((65, 512), F16),        # 2*wk + 2*bk row
    "wv":    ((65, 512), F16),        # wv + bv row
    "wo":    ((64, 8, 64), F16),      # wo per head: [d, h, c]
    "bo":    ((64, 1), F32),
    "lng":   ((128, 64), F32),        # ln_g broadcast over partitions
    "bpos":  ((128, 16, 64), F32),    # ln_b + pos_emb slice, [p, qc, e]
    "lnc":   ((1, 1), F32),           # ln(alpha * SCALE)
    "negs":  ((1, SD), F16),          # constant -1 row
    "onesq": ((1, Q), F16),           # constant +1 row
}
_OUT_NAME = "outt"                    # (64, Q) f32, transposed output


def _emit(ctx: ExitStack, tc: tile.TileContext, io: dict):
    nc = tc.nc
    wp = ctx.enter_context(tc.tile_pool(name="wp", bufs=1))
    work = ctx.enter_context(tc.tile_pool(name="work", bufs=3))
    work2 = ctx.enter_context(tc.tile_pool(name="work2", bufs=2))
    work1 = ctx.enter_context(tc.tile_pool(name="work1", bufs=1))
    pers = ctx.enter_context(tc.tile_pool(name="pers", bufs=1))

    def load(name, shape=None, dt=None, rearrange=None):
        shp, dtt = _IN_SPECS[name]
        t = wp.tile(list(shape or shp), dt or dtt, tag=name, name=name)
        src = io[name]
        if rearrange:
            src = src.rearrange(rearrange)
        nc.sync.dma_start(t[:], src)
        return t

    cwt = load("cwt")
    desct = load("desct", shape=(128, 2, SD), rearrange="c p s -> p c s")
    wdc = load("wdc", shape=(128, 2, 64), rearrange="c p s -> p c s")
    bd = load("bd")
    wq = load("wq")
    wk2 = load("wk2")
    wv = load("wv")
    wo = load("wo")
    bo = load("bo")
    lng = load("lng")
    bpos = load("bpos")
    lnc = load("lnc")

    xt = wp.tile([65, Q], F16, tag="xt")
    nc.sync.dma_start(xt[:64, :], io["xt"])
    nc.sync.dma_start(xt[64:65, :], io["onesq"])

    ident = wp.tile([128, 128], F16, tag="ident")
    make_identity(nc, ident[:])
    ones64 = wp.tile([64, 1], F16, tag="ones64")
    nc.vector.memset(ones64[:], 1.0)

    det = pers.tile([65, SD], F16, tag="det")
    emb = pers.tile([128, 16, 64], F32, tag="emb")
    vemb = pers.tile([128, 16, 64], F16, tag="vemb")
    vet = pers.tile([65, Q], F16, tag="vet")
    vsb = pers.tile([128, 8, 8, 66], F16, tag="vsb")
    qts = [pers.tile([66, Q], F16, tag=f"qt{h}", name=f"qt{h}") for h in range(H)]
    kts = [pers.tile([66, SD], F16, tag=f"kt{h}", name=f"kt{h}") for h in range(H)]
    attn = [pers.tile([66, Q], F16, tag=f"attn{i}", name=f"attn{i}")
            for i in range(H)]
    outt = pers.tile([64, Q], F32, tag="outt")

    # ================= phase A: des_emb, conv, LN, transpose =================
    with tc.tile_pool(name="psA", bufs=2, space="PSUM") as psA:
        ps_de = psA.tile([64, SD], F32, tag="de")
        for sc in range(2):
            for kc in range(2):
                nc.tensor.matmul(
                    ps_de[:, sc * 512:(sc + 1) * 512],
                    lhsT=wdc[:, kc, :], rhs=desct[:, kc, sc * 512:(sc + 1) * 512],
                    start=(kc == 0), stop=(kc == 1))
        nc.scalar.activation(det[:64, :], ps_de[:], ACT.Identity, bias=bd[:])
        nc.sync.dma_start(det[64:65, :], io["onesq"][:, :SD])

        for g in range(4):  # 4 groups of 4 q-chunks of 128
            ps_e = psA.tile([128, 4, 64], F32, tag="e")
            for j in range(4):
                i = g * 4 + j
                nc.tensor.matmul(ps_e[:, j, :], lhsT=xt[:, i * 128:(i + 1) * 128],
                                 rhs=cwt[:], start=True, stop=True)
            nc.vector.tensor_copy(emb[:, g * 4:(g + 1) * 4, :], ps_e[:])

        s1 = work1.tile([128, 16], F32, tag="s1")
        s2 = work1.tile([128, 16], F32, tag="s2")
        sqe = work1.tile([128, 16, 64], F32, tag="sqe")
        nc.scalar.activation(sqe[:], emb[:], ACT.Square)
        nc.vector.tensor_reduce(s1[:], emb[:], axis=AX.X, op=OP.add)
        nc.vector.tensor_reduce(s2[:], sqe[:], axis=AX.X, op=OP.add)
        mu = work1.tile([128, 16], F32, tag="mu")
        nc.vector.tensor_scalar_mul(mu[:], s1[:], 1.0 / 64)
        var = work1.tile([128, 16], F32, tag="var")
        nc.vector.tensor_scalar_mul(var[:], s2[:], 1.0 / 64)
        musq = work1.tile([128, 16], F32, tag="musq")
        nc.vector.tensor_tensor(musq[:], mu[:], mu[:], op=OP.mult)
        nc.vector.tensor_tensor(var[:], var[:], musq[:], op=OP.subtract)
        nc.vector.tensor_scalar_add(var[:], var[:], LN_EPS)
        std = work1.tile([128, 16], F32, tag="std")
        nc.scalar.activation(std[:], var[:], ACT.Sqrt)
        rstd = work1.tile([128, 16], F32, tag="rstd")
        nc.vector.reciprocal(rstd[:], std[:])

        t1 = work1.tile([128, 16, 64], F32, tag="t1")
        nc.vector.tensor_tensor(
            t1[:], emb[:], mu[:, :, None].to_broadcast((128, 16, 64)),
            op=OP.subtract)
        nc.vector.tensor_tensor(
            t1[:], t1[:], rstd[:, :, None].to_broadcast((128, 16, 64)), op=OP.mult)
        nc.vector.tensor_tensor(
            t1[:], t1[:], lng[:, None, :].to_broadcast((128, 16, 64)), op=OP.mult)
        nc.vector.tensor_tensor(vemb[:], t1[:], bpos[:], op=OP.add)

        for qc in range(4):
            ps_t = psA.tile([64, 512], F16, tag="tr")
            for j in range(4):
                i = qc * 4 + j
                nc.tensor.transpose(ps_t[:, j * 128:(j + 1) * 128], vemb[:, i, :],
                                    ident[:])
            nc.vector.tensor_copy(vet[:64, qc * 512:(qc + 1) * 512], ps_t[:])
        nc.sync.dma_start(vet[64:65, :], io["onesq"])

    # ================= phase B: per-head q/k/v prep =================
    nc.gpsimd.memset(vsb[:, :, :, 64:65], 1.0)
    with tc.tile_pool(name="psB", bufs=1, space="PSUM") as psB, \
         tc.tile_pool(name="psB2", bufs=2, space="PSUM") as psB2:
        for h in range(H):
            qt, kt = qts[h], kts[h]
            ps_q = psB.tile([128, Q], F32, tag="q")
            for qc in range(4):
                nc.tensor.matmul(ps_q[:64, qc * 512:(qc + 1) * 512],
                                 lhsT=wq[:, h * 64:(h + 1) * 64],
                                 rhs=vet[:, qc * 512:(qc + 1) * 512],
                                 start=True, stop=True)
            nc.scalar.copy(qt[:64, :], ps_q[:64, :])
            sq = work2.tile([64, Q], F16, tag="sq_q")
            nc.scalar.activation(sq[:], ps_q[:64, :], ACT.Square)
            for qc in range(4):
                ps_q2 = psB2.tile([128, 512], F32, tag="sm")
                nc.tensor.matmul(ps_q2[:1, :], lhsT=ones64[:],
                                 rhs=sq[:, qc * 512:(qc + 1) * 512],
                                 start=True, stop=True)
                nc.vector.tensor_copy(qt[64:65, qc * 512:(qc + 1) * 512],
                                      ps_q2[:1, :])
            nc.sync.dma_start(qt[65:66, :], io["onesq"])

            ps_k = psB.tile([128, SD], F32, tag="k")
            for sc in range(2):
                nc.tensor.matmul(ps_k[:64, sc * 512:(sc + 1) * 512],
                                 lhsT=wk2[:, h * 64:(h + 1) * 64],
                                 rhs=det[:, sc * 512:(sc + 1) * 512],
                                 start=True, stop=True)
            nc.scalar.copy(kt[:64, :], ps_k[:64, :])
            sqk = work2.tile([64, SD], F16, tag="sq_k")
            nc.scalar.activation(sqk[:], ps_k[:64, :], ACT.Square)
            k2row = work2.tile([1, SD], F16, tag="k2row")
            for sc in range(2):
                ps_k2 = psB2.tile([128, 512], F32, tag="sm")
                nc.tensor.matmul(ps_k2[:1, :], lhsT=ones64[:],
                                 rhs=sqk[:, sc * 512:(sc + 1) * 512],
                                 start=True, stop=True)
                nc.vector.tensor_scalar(k2row[:, sc * 512:(sc + 1) * 512],
                                        ps_k2[:1, :], -0.25, lnc[:],
                                        op0=OP.mult, op1=OP.add)
            nc.sync.dma_start(kt[64:65, :], io["negs"])
            nc.sync.dma_start(kt[65:66, :], k2row[:])

            ps_v = psB2.tile([128, 512], F32, tag="sm", name="ps_v")
            nc.tensor.matmul(ps_v[:], lhsT=det[:, h * 128:(h + 1) * 128], rhs=wv[:],
                             start=True, stop=True)
            nc.any.tensor_copy(vsb[:, h, :, 0:64], ps_v[:])

    # ================= phase C: attention main loop + out-proj =================
    with tc.tile_pool(name="psC", bufs=1, space="PSUM") as psC:
        for h in range(H):
            at = psC.tile([128, Q], F32, tag="at")
            for kc in range(8):
                x = psC.tile([128, Q], F32, tag="x")
                for qc in range(4):
                    nc.tensor.matmul(x[:, qc * 512:(qc + 1) * 512],
                                     lhsT=kts[h][:, kc * 128:(kc + 1) * 128],
                                     rhs=qts[h][:, qc * 512:(qc + 1) * 512],
                                     start=True, stop=True)
                u = work.tile([128, Q], F16, tag="u")
                nc.scalar.activation(u[:], x[:], ACT.Exp)
                p = work.tile([128, Q], F16, tag="p")
                nc.scalar.activation(p[:], u[:], ACT.Exp)
                for qc in range(4):
                    nc.tensor.matmul(at[:65, qc * 512:(qc + 1) * 512],
                                     lhsT=vsb[:, kc, h, 0:65],
                                     rhs=p[:, qc * 512:(qc + 1) * 512],
                                     start=(kc == 0), stop=(kc == 7))
            nc.vector.tensor_copy(attn[h][0:65, :], at[0:65, :])
            srec = work2.tile([1, Q], F16, tag="srec")
            with nc.allow_low_precision(reason="1/S fp16: S~1e3, rel 5e-4 ok"):
                nc.vector.reciprocal(srec[:], attn[h][64:65, :])
            recb = work2.tile([64, Q], F16, tag="recb")
            nc.gpsimd.partition_broadcast(recb[:], srec[:])
            nc.vector.tensor_tensor(attn[h][0:64, :], attn[h][0:64, :], recb[:],
                                    op=OP.mult)

        ps_o = psC.tile([128, Q], F32, tag="at")
        for h in range(H):
            for qc in range(4):
                nc.tensor.matmul(ps_o[:64, qc * 512:(qc + 1) * 512],
                                 lhsT=wo[:, h, :],
                                 rhs=attn[h][0:64, qc * 512:(qc + 1) * 512],
                                 start=(h == 0), stop=(h == 7))
        nc.scalar.activation(outt[:], ps_o[:64, :], ACT.Identity, bias=bo[:])
    nc.sync.dma_start(io[_OUT_NAME], outt[:])


def build_module():
    nc = bacc.Bacc("TRN2", target_bir_lowering=False, debug=False,
                   enable_asserts=False, num_devices=NCORES)
    io = {}
    for name, (shape, dt) in _IN_SPECS.items():
        io[name] = nc.dram_tensor(name, list(shape), dt, kind="ExternalInput").ap()
    io[_OUT_NAME] = nc.dram_tensor(_OUT_NAME, [64, Q], F32, kind="ExternalOutput").ap()
    with tile.TileContext(nc) as tc:
        with ExitStack() as ctx:
            _emit(ctx, tc, io)
    nc.compile()
    return nc


def shard_inputs(inputs) -> list:
    v = {k: np.asarray(val) for k, val in inputs.items()}
    f32 = np.float32
    f16 = np.float16
    cw = v["conv_w"].reshape(E, 64).astype(f32)
    cwt = np.concatenate([cw.T, v["conv_b"][None, :]], 0).astype(f16)     # [65, 64]
    wdc = np.ascontiguousarray(v["wd"].reshape(2, 128, 64)).astype(f16)   # [2,128,64]
    wq = np.concatenate([v["wq"], v["bq"][None, :]], 0).astype(f16)       # [65, 512]
    wk2 = np.concatenate([2.0 * v["wk"], 2.0 * v["bk"][None, :]], 0).astype(f16)
    wv = np.concatenate([v["wv"], v["bv"][None, :]], 0).astype(f16)
    wo = np.ascontiguousarray(
        v["wo"].reshape(8, 64, 64).transpose(1, 0, 2)).astype(f16)   # [d, h, c]
    bo = v["bo"].reshape(64, 1).astype(f32)
    bd = v["bd"].reshape(64, 1).astype(f32)
    lng = np.broadcast_to(v["ln_g"], (128, 64)).astype(f32).copy()
    lnc = np.array([[np.log(np.float32(v["alpha"]) * SCALE)]], dtype=f32)

    maps = []
    for c in range(NCORES):
        b, qh = c // 2, c % 2
        vox = v["vox_features"][b, 0].reshape(16, 4, 16, 4, 16, 4)
        X = vox.transpose(0, 2, 4, 1, 3, 5).reshape(NP_, 64)[qh * Q:(qh + 1) * Q]
        xt = np.ascontiguousarray(X.T).astype(f16)                        # [64, Q]
        desct = np.ascontiguousarray(v["descriptor"][b].T).reshape(2, 128, SD).astype(f16)
        bp = (v["ln_b"][None, :] + v["pos_emb"][0, qh * Q:(qh + 1) * Q, :]).astype(f32)
        bpos = np.ascontiguousarray(bp.reshape(16, 128, 64).transpose(1, 0, 2))
        maps.append({
            "xt": xt, "cwt": cwt, "desct": desct, "wdc": wdc, "bd": bd,
            "wq": wq, "wk2": wk2, "wv": wv, "wo": wo, "bo": bo,
            "lng": lng, "bpos": bpos, "lnc": lnc,
            "negs": np.full((1, SD), -1.0, np.float16),
            "onesq": np.ones((1, Q), np.float16),
        })
    return maps


def assemble_output(results) -> np.ndarray:
    out = np.zeros((B, NP_, 64), np.float32)
    for c in range(NCORES):
        b, qh = c // 2, c % 2
        out[b, qh * Q:(qh + 1) * Q, :] = results[c][_OUT_NAME].T
    return out


_NC_CACHE = {}


def kernel(**inputs) -> np.ndarray:
    if "nc" not in _NC_CACHE:
        _NC_CACHE["nc"] = build_module()
    nc = _NC_CACHE["nc"]
    maps = shard_inputs(inputs)
    res = run_bass_kernel_spmd(nc, maps, core_ids=list(range(NCORES)))
    return assemble_output(res.results)
